# revision 48
# baseline (speedup 1.0000x reference)
"""Trainium2 Bass kernel for nn_CrossAttention (sparse_attention).

Per head h (one NeuronCore per head), with e1=emb_1, e2=emb_2, D=256:
  q_g = e_g Wq + bq ; k_g = e_g Wk + bk
  a_1 = SCALE * (q_1 k_2^T) * mask_1     mask_1[i,j] = nt1[i]==nt2[j]
  a_2 = SCALE * (k_1 q_2^T) * mask_2     mask_2 = mask_1^T

Algebraic restructure (exact):
  a_1 = e1 M2 + 1 c^T    with M2 = SCALE*(G e2^T + g 1^T), G = Wq Wk^T,
                              g = Wq bk, c = SCALE*(e2 (Wk bq) + bq.bk)
  a_2 = e1 M2' + 1 c'^T  with M2' = SCALE*(G^T e2^T + w 1^T), w = Wk bq,
                              c' = SCALE*(e2 (Wq bk) + bq.bk)
so the device does: 2 projections (G e2T: 256x256x2048 each) and the
block-diagonal score matmuls, all in fp16 operands (fp32 PSUM accumulation).
Scores are computed TRANSPOSED (a_1^T = M2^T e1^T + c 1^T) so the rank-1
bias c is per-PARTITION and fuses into the PSUM->SBUF copy (ACT activation
bias / DVE tensor_scalar_add). SCALE + bias algebra fold host-side into
G/g/w/c (O(N*D) prep, same class as the host sort/transpose).

Node-type sort (host) makes each masked score matrix block-diagonal: only
the 5 matching-type blocks are computed. Blocks are written exactly packed
([rows_t, w_t] row-major) to a flat DRAM output with 2 DMAs per block
(full 128-row tiles + remainder rows) -- each dma_start costs ~625ns on
the single shared HWDGE device, so DMA count is minimized and inputs are
loaded in column chunks sized to overlap the PE pipeline startup. The
host scatters blocks into the full [16, N, N] fp32 output.
"""

import numpy as np

N = 2048
D = 256
H = 8
T = 5
SCALE = D ** (-0.5)
NCORES = 8
P = 128

NWARM = 9           # PE p-state warmup matmuls (cover first input DMA latency)
CHUNK = 512         # score matmul moving-dim chunk (PSUM bank = 512 fp32)

_PROG_CACHE: dict = {}


def _plan(c1, c2):
    """Static schedule: blocks (emission order), row tiles, packed offsets.

    Matrix 0 holds a_1^T: partition rows = perm2-sorted (sizes c2), moving
    cols = perm1-sorted (sizes c1). Matrix 1 holds a_2^T: rows = perm1-
    sorted (sizes c1), cols = perm2-sorted (sizes c2).

    PE matmul cost is moving-dim columns only, so a partial row tile
    (rem < 128 rows) streaming the full block width w wastes w*2 cycles.
    When cheaper, that remainder is computed FLIPPED (raw e-tensor
    stationary, M2 moving): ceil(w/P) passes of rem columns, plus a K=1
    ones-row matmul that accumulates the (now per-column) bias into PSUM.
    The flipped piece is written padded ([ceil(w/P)*P, rem] row-major).
    """
    def bounds(cnt):
        b = [0]
        for c in cnt:
            b.append(b[-1] + int(c))
        return b

    b1, b2 = bounds(c1), bounds(c2)

    raw = {0: [], 1: []}
    for mat, (rbv, cbv) in ((0, (b2, b1)), (1, (b1, b2))):
        for t in range(T):
            rows_tot = rbv[t + 1] - rbv[t]
            w = cbv[t + 1] - cbv[t]
            if rows_tot and w:
                rem = rows_tot % P
                gp = (w + P - 1) // P
                flip = rem > 0 and rem * (2 * gp + 1) < 2 * w
                raw[mat].append((t, rbv[t], rows_tot, cbv[t], w, flip))

    # the kernel ends on matrix 1's last block: prefer the cheapest final
    # unit (flipped piece bytes, else remainder rows, else a full tile)
    def tailcost(blk):
        _, _, rows_tot, _, w, flip = blk
        rem = rows_tot % P
        if flip:
            return ((w + P - 1) // P) * P * rem
        return (rem if rem else P) * w

    if raw[1]:
        last = min(raw[1], key=tailcost)
        raw[1] = [b for b in raw[1] if b is not last] + [last]

    # block: (mat, t, r0s, rows_tot, c0, w, g, gf, rem, off, flip, poff, gp)
    blocks = []
    tiles = []           # main-part tiles only: (mat, t, gi, r0, rows, tau)
    off = 0
    for mat in (0, 1):
        for (t, r0s, rows_tot, c0, w, flip) in raw[mat]:
            gf, rem = divmod(rows_tot, P)
            g = gf + (1 if (rem and not flip) else 0)
            gp = (w + P - 1) // P if flip else 0
            boff = off
            off += (gf * P if flip else rows_tot) * w
            poff = off
            if flip:
                off += gp * P * rem
            blocks.append((mat, t, r0s, rows_tot, c0, w, g, gf, rem, boff,
                           flip, poff, gp))
            for gi in range(g):
                rows = min(P, rows_tot - gi * P)
                tiles.append((mat, t, gi, r0s + gi * P, rows, len(tiles)))
    return b1, b2, blocks, tiles, off


def _build_program(c1: tuple, c2: tuple):
    import concourse.bass as bass  # noqa: F401
    import concourse.mybir as mybir
    import concourse.tile as tile
    from concourse import bacc

    f32 = mybir.dt.float32
    f16 = mybir.dt.float16
    AF = mybir.ActivationFunctionType

    b1, b2, blocks, tiles, out_tot = _plan(c1, c2)
    ntiles = len(tiles)
    F = 4 + ntiles  # fpar cols: gs0, gs1, ws0, ws1, then per-row-tile bias

    nc = bacc.Bacc("TRN2", target_bir_lowering=False, debug=False,
                   num_devices=NCORES)

    e_dram = {
        v: nc.dram_tensor(v, [D, N], f16, kind="ExternalInput")
        for v in ("e1p1", "e1p2", "e2p1", "e2p2")
    }
    gq_d = nc.dram_tensor("gq", [D, D], f16, kind="ExternalInput")
    gk_d = nc.dram_tensor("gk", [D, D], f16, kind="ExternalInput")
    fpar_d = nc.dram_tensor("fpar", [P, F], f32, kind="ExternalInput")
    crow_d = nc.dram_tensor("crow", [2, N], f16, kind="ExternalInput")
    out_d = nc.dram_tensor("out", [out_tot], f16, kind="ExternalOutput")

    with tile.TileContext(nc) as tc:
        with (
            tc.tile_pool(name="const", bufs=1) as constp,
            tc.tile_pool(name="stage", bufs=1) as stagep,
            tc.tile_pool(name="pproj", bufs=3, space="PSUM") as psum_p,
            tc.tile_pool(name="pscore", bufs=5, space="PSUM") as psum_s,
        ):
            # ---- SBUF tiles ----
            gq = constp.tile([P, 2, D], f16, tag="gq")
            gk = constp.tile([P, 2, D], f16, tag="gk")
            fpar = constp.tile([P, F], f32, tag="fpar")
            esb = {v: constp.tile([P, 2, N], f16, tag=v, name=v)
                   for v in ("e1p1", "e1p2", "e2p1", "e2p2")}
            m2sb = [constp.tile([P, 2, N], f16, tag=f"m2_{m}", name=f"m2_{m}")
                    for m in range(2)]
            crow = constp.tile([1, 2, N], f16, tag="crow")
            ones = constp.tile([1, P], f16, tag="ones")
            stage = {}
            pstage = {}
            for (mat, t, r0s, rows_tot, c0, w, g, gf, rem, off,
                 flip, poff, gp) in blocks:
                if g:
                    stage[(mat, t)] = stagep.tile(
                        [P, g, w], f16, tag=f"st{mat}_{t}", name=f"st{mat}_{t}")
                if flip:
                    pst = stagep.tile([P, gp, rem], f16, tag=f"pt{mat}_{t}",
                                      name=f"pt{mat}_{t}")
                    pstage[(mat, t)] = pst
                    prow = w % P
                    if prow:
                        # pad rows of the piece's last pass: host ignores them
                        nc.gpsimd.memset(pst[prow:P, gp - 1, :], 0.0)

            # ---- input DMAs, issue order == need order, chunked ----
            def load(tl, dram, j0, j1):
                nc.sync.dma_start(
                    tl[:, :, j0:j1],
                    dram.ap().rearrange("(c p) n -> p c n", p=P)[:, :, j0:j1],
                )

            E2CUTS = [0, 512, 1024, 1536, 2048]
            nc.sync.dma_start(gq[:], gq_d.ap().rearrange("(c p) e -> p c e", p=P))
            load(esb["e2p2"], e_dram["e2p2"], E2CUTS[0], E2CUTS[1])
            load(esb["e2p2"], e_dram["e2p2"], E2CUTS[1], E2CUTS[2])
            nc.sync.dma_start(fpar[:], fpar_d[:, :])
            for j in range(2, 4):
                load(esb["e2p2"], e_dram["e2p2"], E2CUTS[j], E2CUTS[j + 1])
            for j in range(2):
                load(esb["e1p1"], e_dram["e1p1"], j * 512, (j + 1) * 512)
            nc.sync.dma_start(crow[:], crow_d.ap().rearrange("(o c) n -> o c n", o=1))
            for j in range(2, 4):
                load(esb["e1p1"], e_dram["e1p1"], j * 512, (j + 1) * 512)
            nc.sync.dma_start(gk[:], gk_d.ap().rearrange("(c p) e -> p c e", p=P))
            for j in range(2):
                load(esb["e2p1"], e_dram["e2p1"], j * 1024, (j + 1) * 1024)
            for j in range(2):
                load(esb["e1p2"], e_dram["e1p2"], j * 1024, (j + 1) * 1024)

            # ---- PE p-state warmup on a zero tile (fills DMA latency) ----
            # split memset so the first (narrow) warmup matmuls start ASAP
            wt = constp.tile([P, 512], f16, tag="wt")
            nc.vector.memset(wt[:, 0:P], 0.0)
            nc.vector.memset(wt[:, P:512], 0.0)
            nc.vector.memset(ones[:], 1.0)
            for i in range(NWARM):
                wps = psum_p.tile([P, 512], f32, tag="pp", name="pp")
                rhs = wt[:, 0:P] if i < 2 else wt[:]
                nc.tensor.matmul(wps[0:P, 0:rhs.shape[-1]], wt[:, 0:P], rhs,
                                 start=True, stop=True)

            cp = 0  # ACT/DVE alternation counter

            def copy_bias(dst, src, bias):
                nonlocal cp
                if cp % 2 == 0:
                    nc.scalar.activation(dst, src, AF.Identity, bias=bias, scale=1.0)
                else:
                    nc.vector.tensor_scalar_add(dst, src, bias)
                cp += 1

            # ---- projections: M2 = gq.T @ e2T (+gs), M2' = gk.T @ e2T (+ws) ----
            def proj(mat, src, gt, bcol, cuts):
                for j2 in range(len(cuts) - 1):
                    x0, x1 = cuts[j2], cuts[j2 + 1]
                    cw = x1 - x0
                    for m in range(2):
                        ps = psum_p.tile([P, 512], f32, tag="pp", name="pp")
                        for c in range(2):
                            nc.tensor.matmul(
                                ps[0:P, 0:cw],
                                gt[:, c, m * P:(m + 1) * P],
                                src[:, c, x0:x1],
                                start=(c == 0), stop=(c == 1),
                            )
                        copy_bias(
                            m2sb[mat][:, m, x0:x1],
                            ps[0:P, 0:cw], fpar[:, bcol + m:bcol + m + 1],
                        )

            # ---- block-diagonal scores (output transposed, exact-packed) ----
            tau_of = {(mt, tt, gg): tau for (mt, tt, gg, _, _, tau) in tiles}

            def plain_copy(dst, src):
                nonlocal cp
                if cp % 2 == 0:
                    nc.scalar.copy(dst, src)
                else:
                    nc.vector.tensor_copy(dst, src)
                cp += 1

            def scores(mat, rhs_name):
                rhs = esb[rhs_name]
                mblocks = [b for b in blocks if b[0] == mat]
                for bi, (bmat, t, r0s, rows_tot, c0, w, g, gf, rem, off,
                         flip, poff, gp) in enumerate(mblocks):
                    # final blocks: split the full-part DMA so transfers start
                    # before the last tile's copies land
                    is_last = (mat == 1 and bi >= len(mblocks) - 2)
                    st = stage.get((mat, t))
                    nch = (w + CHUNK - 1) // CHUNK
                    cuts = np.linspace(0, w, nch + 1).astype(int)

                    def full_dma(g0, g1):
                        # big packed DMAs go out via Pool/SWDGE (bypasses the
                        # near-saturated shared HWDGE device) -- except at the
                        # very end, where HWDGE is free and issues 2x faster
                        eng = nc.sync if (is_last and bi == len(mblocks) - 1) else nc.gpsimd
                        eng.dma_start(
                            out_d[off + g0 * P * w:off + g1 * P * w].rearrange(
                                "(g p w) -> p g w", p=P, w=w),
                            st[:, g0:g1, :],
                        )

                    if flip:
                        # remainder rows, flipped: stationary = raw e slice,
                        # moving = M2 tail columns (rem wide); bias via a K=1
                        # ones-row matmul accumulating crow into PSUM
                        pst = pstage[(mat, t)]
                        m0 = r0s + gf * P
                        # one PSUM bank for the whole piece: pass pi lands in
                        # cols [pi*rem, (pi+1)*rem); a single copy drains it
                        # (pad rows carry garbage the host never reads)
                        ps = psum_s.tile([P, 512], f32, tag="ss", name="ss")
                        for pi in range(gp):
                            i0 = c0 + pi * P
                            irows = min(P, c0 + w - i0)
                            x0 = pi * rem
                            for c in range(2):
                                nc.tensor.matmul(
                                    ps[0:irows, x0:x0 + rem],
                                    rhs[:, c, i0:i0 + irows],
                                    m2sb[mat][:, c, m0:m0 + rem],
                                    start=(c == 0), stop=False,
                                )
                            nc.tensor.matmul(
                                ps[0:irows, x0:x0 + rem],
                                ones[0:1, 0:irows],
                                crow[0:1, mat, m0:m0 + rem],
                                start=False, stop=True,
                            )
                        plain_copy(pst[:].rearrange("p g w -> p (g w)"),
                                   ps[:, 0:gp * rem])
                        peng = nc.sync
                        peng.dma_start(
                            out_d[poff:poff + gp * P * rem].rearrange(
                                "(g p w) -> p g w", p=P, w=rem),
                            pst[:],
                        )
                    for gi in range(g):
                        r0 = r0s + gi * P
                        rows = min(P, r0s + rows_tot - r0)
                        tau = tau_of[(mat, t, gi)]
                        for ci in range(nch):
                            x0, x1 = int(cuts[ci]), int(cuts[ci + 1])
                            cw = x1 - x0
                            ps = psum_s.tile([P, 512], f32, tag="ss", name="ss")
                            for c in range(2):
                                nc.tensor.matmul(
                                    ps[0:rows, 0:cw],
                                    m2sb[mat][:, c, r0:r0 + rows],
                                    rhs[:, c, c0 + x0:c0 + x1],
                                    start=(c == 0), stop=(c == 1),
                                )
                            copy_bias(
                                st[0:rows, gi, x0:x1],
                                ps[0:rows, 0:cw],
                                fpar[0:rows, 4 + tau:5 + tau],
                            )
                        if is_last and gf > 1 and gi == gf - 2:
                            full_dma(0, gf - 1)
                        elif is_last and gi == gf - 1:
                            full_dma(gf - 1, gf)
                        elif not is_last and gi == gf - 1:
                            full_dma(0, gf)
                    if rem and not flip:
                        nc.sync.dma_start(
                            out_d[off + gf * P * w:off + rows_tot * w].rearrange(
                                "(p w) -> p w", w=w),
                            st[0:rem, gf, :],
                        )
            proj(0, esb["e2p2"], gq, 0, E2CUTS)
            scores(0, "e1p1")
            proj(1, esb["e2p1"], gk, 2, [0, 512, 1024, 1536, 2048])
            scores(1, "e1p2")

    nc.compile()
    return nc


def _get_program(c1, c2):
    key = (tuple(int(x) for x in c1), tuple(int(x) for x in c2))
    if key not in _PROG_CACHE:
        _PROG_CACHE[key] = _build_program(key[0], key[1])
    return _PROG_CACHE[key]


def kernel(emb_1, emb_2, node_type_1, node_type_2, W_q, b_q, W_k, b_k):
    from concourse.bass_utils import run_bass_kernel_spmd

    e1 = np.asarray(emb_1, dtype=np.float64)
    e2 = np.asarray(emb_2, dtype=np.float64)
    nt1 = np.asarray(node_type_1).astype(np.int64)
    nt2 = np.asarray(node_type_2).astype(np.int64)
    W_q = np.asarray(W_q, dtype=np.float64)
    W_k = np.asarray(W_k, dtype=np.float64)
    b_q = np.asarray(b_q, dtype=np.float64)
    b_k = np.asarray(b_k, dtype=np.float64)

    perm1 = np.argsort(nt1, kind="stable")
    perm2 = np.argsort(nt2, kind="stable")
    c1 = np.bincount(nt1, minlength=T)
    c2 = np.bincount(nt2, minlength=T)

    nc = _get_program(c1, c2)
    b1, b2, blocks, tiles, out_tot = _plan(tuple(c1), tuple(c2))
    F = 4 + len(tiles)

    e1T = e1.T.astype(np.float16)   # [D, N]
    e2T = e2.T.astype(np.float16)
    ins_shared = {
        "e1p1": np.ascontiguousarray(e1T[:, perm1]),
        "e1p2": np.ascontiguousarray(e1T[:, perm2]),
        "e2p1": np.ascontiguousarray(e2T[:, perm1]),
        "e2p2": np.ascontiguousarray(e2T[:, perm2]),
    }

    in_maps = []
    for h in range(NCORES):
        sl = slice(h * D, (h + 1) * D)
        Wq, Wk = W_q[:, sl], W_k[:, sl]
        bq, bk = b_q[sl], b_k[sl]
        G = Wq @ Wk.T                      # [D, D]
        g_v = Wq @ bk                      # [D]
        w_v = Wk @ bq
        s = float(bq @ bk)

        cvec = {
            0: (SCALE * (e2 @ w_v + s))[perm2],   # a1^T row bias (perm2 order)
            1: (SCALE * (e2 @ g_v + s))[perm1],   # a2^T row bias (perm1 order)
        }
        fpar = np.zeros((P, F), dtype=np.float32)
        fpar[:, 0] = SCALE * g_v[0:P]
        fpar[:, 1] = SCALE * g_v[P:2 * P]
        fpar[:, 2] = SCALE * w_v[0:P]
        fpar[:, 3] = SCALE * w_v[P:2 * P]
        for (mat, t, gi, r0, rows, tau) in tiles:
            fpar[0:rows, 4 + tau] = cvec[mat][r0:r0 + rows]

        im = dict(ins_shared)
        im["gq"] = np.ascontiguousarray((SCALE * G.T).astype(np.float16))
        im["gk"] = np.ascontiguousarray((SCALE * G).astype(np.float16))
        im["fpar"] = fpar
        im["crow"] = np.stack([cvec[0], cvec[1]]).astype(np.float16)
        in_maps.append(im)

    res = run_bass_kernel_spmd(nc, in_maps, core_ids=list(range(NCORES)))

    out = np.zeros((2 * H, N, N), dtype=np.float32)
    segs1 = [perm1[b1[t]:b1[t + 1]] for t in range(T)]
    segs2 = [perm2[b2[t]:b2[t + 1]] for t in range(T)]
    for h in range(NCORES):
        packed = np.asarray(res.results[h]["out"]).astype(np.float32)
        for (mat, t, r0s, rows_tot, c0, w, g, gf, rem, off,
             flip, poff, gp) in blocks:
            mrows = gf * P if flip else rows_tot
            rseg = segs2[t] if mat == 0 else segs1[t]
            cseg = segs1[t] if mat == 0 else segs2[t]
            dst = out[h] if mat == 0 else out[H + h]
            if mrows:
                blk = packed[off:off + mrows * w].reshape(mrows, w)
                dst[cseg[None, :], rseg[:mrows, None]] = blk
            if flip:
                pc = packed[poff:poff + gp * P * rem].reshape(gp * P, rem)[:w]
                dst[cseg[:, None], rseg[mrows:, None].T] = pc
    return out
